# revision 2
# baseline (speedup 1.0000x reference)
"""Trainium2 Bass kernel for nn_DemLocDecoder (GIN message passing + classifier).

8-way tensor-parallel, all column-parallel layers with AllGather of the tiny
[*, 19] transposed activations between layers.  Activations are kept
feature-major ("transposed", [D, 19]) throughout, so every layer is a
weight-stationary matmul (lhsT = weight tile [128,128], rhs = activation
tile [128,19]) and biases become per-partition vectors applied by the
scalar-engine activation instruction.  The graph aggregation (I+A) @ h is
folded through matmul associativity so it is applied to the narrow
[19, 512] per-core tensor in node-major form, between two PE transposes.

Sharding (core i of 8):
  W1a[:, i*256:(i+1)*256], W1b[:, i*256:(i+1)*256]   (column shards)
  W2a[:, i*512:(i+1)*512], W2b[:, i*512:(i+1)*512]
  Wc1 rows (n, t) with t in [i*512, (i+1)*512)       (row shard, matches
                                                      the L2b column shard)
Per-core outputs: reconT slice [512, 19] and the partial classifier logit.
Host unshard: concat reconT slices -> [4096,19] -> transpose; sum the 8
partial logits and apply the (scalar) sigmoid.
"""

import numpy as np
import ml_dtypes
from contextlib import ExitStack

import concourse.bass as bass
import concourse.tile as tile
from concourse import bacc, mybir
from concourse import bass_utils
from concourse.masks import make_identity

NCORES = 8
NN = 19          # nodes
LATENT = 512
HID = 2048
NT = 4096
P = 128

C1 = HID // NCORES    # 256  (layer-1 column shard)
C2 = NT // NCORES     # 512  (layer-2 column shard)
RCLS = NN * C2        # 9728 (classifier row shard)

# dtype config: trunk (GIN layers) and classifier (Wc1 path)
DT_TRUNK = mybir.dt.float32
DT_CLS = mybir.dt.float32
NP_TRUNK = np.float32
NP_CLS = np.float32


def _dt_np(dt):
    return {mybir.dt.float32: np.float32,
            mybir.dt.bfloat16: ml_dtypes.bfloat16,
            mybir.dt.float8e4: ml_dtypes.float8_e4m3}[dt]


def build(reps: int = 1):
    """Build (and compile) the 8-core SPMD Bass program.

    reps > 1 repeats the whole body (including weight DMA) for timing by
    deltas; outputs are simply overwritten each repetition.
    """
    nc = bacc.Bacc("TRN2", target_bir_lowering=False, debug=False,
                   enable_asserts=True, num_devices=NCORES)
    f32 = mybir.dt.float32

    # ---- kernel I/O (per-core shapes; values differ per core) ----
    zin = nc.dram_tensor("zin", [NN, LATENT], DT_TRUNK, kind="ExternalInput")
    ahatT = nc.dram_tensor("ahatT", [NN, NN], DT_TRUNK, kind="ExternalInput")
    w1a = nc.dram_tensor("w1a", [LATENT, C1], DT_TRUNK, kind="ExternalInput")
    w1b = nc.dram_tensor("w1b", [HID, C1], DT_TRUNK, kind="ExternalInput")
    w2a = nc.dram_tensor("w2a", [HID, C2], DT_TRUNK, kind="ExternalInput")
    w2b = nc.dram_tensor("w2b", [NT, C2], DT_TRUNK, kind="ExternalInput")
    wc1 = nc.dram_tensor("wc1", [RCLS, LATENT], DT_CLS, kind="ExternalInput")
    b1a = nc.dram_tensor("b1a", [P, C1 // P], f32, kind="ExternalInput")
    b1b = nc.dram_tensor("b1b", [P, C1 // P], f32, kind="ExternalInput")
    b2a = nc.dram_tensor("b2a", [1, C2], DT_TRUNK, kind="ExternalInput")
    b2b = nc.dram_tensor("b2b", [P, C2 // P], f32, kind="ExternalInput")
    wc2 = nc.dram_tensor("wc2", [1, LATENT], f32, kind="ExternalInput")
    beta = nc.dram_tensor("beta", [1, 1], f32, kind="ExternalInput")

    recont_out = nc.dram_tensor("recont_out", [C2, NN], f32, kind="ExternalOutput")
    c2_out = nc.dram_tensor("c2_out", [1, 1], f32, kind="ExternalOutput")

    K1 = LATENT // P   # 4
    KH = HID // P      # 16
    KT = NT // P       # 32
    M1 = C1 // P       # 2
    M2 = C2 // P       # 4

    with tile.TileContext(nc) as tc, ExitStack() as ctx:
        const = ctx.enter_context(tc.tile_pool(name="const", bufs=1))
        wpool = ctx.enter_context(tc.tile_pool(name="wpool", bufs=1))
        wc1p = ctx.enter_context(tc.tile_pool(name="wc1p", bufs=6))
        act = ctx.enter_context(tc.tile_pool(name="act", bufs=1))
        ps_s = ctx.enter_context(tc.tile_pool(name="ps_s", bufs=4, space="PSUM"))
        ps_w = ctx.enter_context(tc.tile_pool(name="ps_w", bufs=2, space="PSUM"))
        ps_c = ctx.enter_context(tc.tile_pool(name="ps_c", bufs=1, space="PSUM"))
        dram = ctx.enter_context(tc.tile_pool(name="dram", bufs=1, space="DRAM"))

        # ---- constants / padded node-major buffers (zeroed once) ----
        ident = const.tile([P, P], DT_TRUNK)
        make_identity(nc, ident[:])
        zsb = const.tile([P, LATENT], DT_TRUNK)
        nc.any.memzero(zsb[:])
        ahatT_sb = const.tile([P, NN], DT_TRUNK)
        nc.any.memzero(ahatT_sb[:])
        ones_sb = const.tile([P, NN], DT_TRUNK)
        nc.any.memzero(ones_sb[:])
        nc.gpsimd.memset(ones_sb[:1, :], 1.0)
        b2a_sb = const.tile([P, C2], DT_TRUNK)
        nc.any.memzero(b2a_sb[:])
        h0_sb = const.tile([P, LATENT], DT_TRUNK)
        nc.any.memzero(h0_sb[:])
        v_sb = const.tile([P, C2], DT_TRUNK)
        nc.any.memzero(v_sb[:])
        a2_sb = const.tile([P, C2], DT_TRUNK)
        nc.any.memzero(a2_sb[:])

        b1a_sb = const.tile([P, C1 // P], f32)
        b1b_sb = const.tile([P, C1 // P], f32)
        b2b_sb = const.tile([P, C2 // P], f32)
        wc2_sb = const.tile([1, LATENT], f32)
        beta_sb = const.tile([1, 1], f32)
        nc.sync.dma_start(b1a_sb[:], b1a.ap())
        nc.sync.dma_start(b1b_sb[:], b1b.ap())
        nc.sync.dma_start(b2b_sb[:], b2b.ap())
        nc.sync.dma_start(wc2_sb[:], wc2.ap())
        nc.sync.dma_start(beta_sb[:], beta.ap())

        # ---- persistent weight tiles ----
        w1a_sb = wpool.tile([P, K1, C1], DT_TRUNK)
        w1b_sb = wpool.tile([P, KH, C1], DT_TRUNK)
        w2a_sb = wpool.tile([P, KH, C2], DT_TRUNK)
        w2b_sb = wpool.tile([P, KT, C2], DT_TRUNK)

        # ---- activations ----
        h0T_sb = act.tile([P, K1, NN], DT_TRUNK)
        h1T_loc = act.tile([P, M1, NN], DT_TRUNK)
        h1T_sb = act.tile([P, KH, NN], DT_TRUNK)
        h2T_loc = act.tile([P, M1, NN], DT_TRUNK)
        h2T_sb = act.tile([P, KH, NN], DT_TRUNK)
        vT_sb = act.tile([P, M2, NN], DT_TRUNK)
        a2t_loc = act.tile([P, M2, NN], DT_TRUNK)
        a2T_sb = act.tile([P, KT, NN], DT_TRUNK)
        recont_sb = act.tile([P, M2, NN], f32)
        recont_cls = act.tile([P, M2, NN], DT_CLS) if DT_CLS != f32 else recont_sb
        c1_sb = act.tile([1, LATENT], f32)
        prod_sb = act.tile([1, LATENT], f32)
        c2_sb = act.tile([1, 1], f32)

        # ---- collective bounce buffers ----
        cc1_in = dram.tile([M1, P, NN], DT_TRUNK)
        cc1_out = dram.tile([KH, P, NN], DT_TRUNK)
        cc2_in = dram.tile([M1, P, NN], DT_TRUNK)
        cc2_out = dram.tile([KH, P, NN], DT_TRUNK)
        cc3_in = dram.tile([M2, P, NN], DT_TRUNK)
        cc3_out = dram.tile([KT, P, NN], DT_TRUNK)

        rg = [list(range(NCORES))]

        def allgather(cin, cout):
            nc.gpsimd.collective_compute(
                "AllGather", mybir.AluOpType.bypass, replica_groups=rg,
                ins=[cin.opt()], outs=[cout.opt()])

        for _rep in range(reps):
            # ---- load inputs / weights ----
            nc.sync.dma_start(zsb[:NN, :], zin.ap())
            nc.sync.dma_start(ahatT_sb[:NN, :], ahatT.ap())
            nc.sync.dma_start(b2a_sb[:1, :], b2a.ap())
            nc.sync.dma_start(w1a_sb[:], w1a.ap().rearrange("(o p) m -> p o m", p=P))
            nc.sync.dma_start(w1b_sb[:], w1b.ap().rearrange("(o p) m -> p o m", p=P))
            nc.sync.dma_start(w2a_sb[:], w2a.ap().rearrange("(o p) m -> p o m", p=P))
            nc.sync.dma_start(w2b_sb[:], w2b.ap().rearrange("(o p) m -> p o m", p=P))

            # ---- agg1 (node-major) + transpose to feature-major ----
            ps_h0 = ps_w.tile([P, LATENT], f32, tag="ps_wide")
            nc.tensor.matmul(ps_h0[:NN, :], ahatT_sb[:], zsb[:], start=True, stop=True)
            nc.scalar.copy(h0_sb[:NN, :], ps_h0[:NN, :])
            for s in range(K1):
                ps_t = ps_s.tile([P, NN], f32, tag="ps_small")
                nc.tensor.matmul(ps_t[:], h0_sb[:, bass.ts(s, P)], ident[:, :NN],
                                 start=True, stop=True)
                nc.scalar.copy(h0T_sb[:, s, :], ps_t[:])

            # ---- L1a (column-parallel, weight-stationary) ----
            for mt in range(M1):
                ps = ps_s.tile([P, NN], f32, tag="ps_small")
                for kt in range(K1):
                    nc.tensor.matmul(ps[:], w1a_sb[:, kt, bass.ts(mt, P)],
                                     h0T_sb[:, kt, :],
                                     start=(kt == 0), stop=(kt == K1 - 1))
                nc.scalar.activation(h1T_loc[:, mt, :], ps[:],
                                     mybir.ActivationFunctionType.Relu,
                                     bias=b1a_sb[:, mt:mt + 1])
                nc.sync.dma_start(cc1_in[mt], h1T_loc[:, mt, :])
            allgather(cc1_in, cc1_out)
            nc.sync.dma_start(h1T_sb[:], cc1_out[:].rearrange("o p n -> p o n"))

            # ---- L1b (column-parallel) + inter-layer relu ----
            for mt in range(M1):
                ps = ps_s.tile([P, NN], f32, tag="ps_small")
                for kt in range(KH):
                    nc.tensor.matmul(ps[:], w1b_sb[:, kt, bass.ts(mt, P)],
                                     h1T_sb[:, kt, :],
                                     start=(kt == 0), stop=(kt == KH - 1))
                nc.scalar.activation(h2T_loc[:, mt, :], ps[:],
                                     mybir.ActivationFunctionType.Relu,
                                     bias=b1b_sb[:, mt:mt + 1])
                nc.sync.dma_start(cc2_in[mt], h2T_loc[:, mt, :])
            allgather(cc2_in, cc2_out)
            nc.sync.dma_start(h2T_sb[:], cc2_out[:].rearrange("o p n -> p o n"))

            # ---- L2a:  v = h2 @ W2a_loc  (agg folded to after, by associativity) ----
            for mt in range(M2):
                ps = ps_s.tile([P, NN], f32, tag="ps_small")
                for kt in range(KH):
                    nc.tensor.matmul(ps[:], w2a_sb[:, kt, bass.ts(mt, P)],
                                     h2T_sb[:, kt, :],
                                     start=(kt == 0), stop=(kt == KH - 1))
                nc.scalar.copy(vT_sb[:, mt, :], ps[:])

            # ---- agg2 + bias + relu (node-major), then back to feature-major ----
            ps_vn = ps_w.tile([P, C2], f32, tag="ps_wide")
            for s in range(M2):
                nc.tensor.matmul(ps_vn[:NN, bass.ts(s, P)], vT_sb[:, s, :], ident[:],
                                 start=True, stop=True)
            nc.scalar.copy(v_sb[:NN, :], ps_vn[:NN, :])
            ps_u = ps_w.tile([P, C2], f32, tag="ps_wide")
            nc.tensor.matmul(ps_u[:NN, :], ahatT_sb[:], v_sb[:], start=True, stop=False)
            nc.tensor.matmul(ps_u[:NN, :], ones_sb[:], b2a_sb[:], start=False, stop=True)
            nc.scalar.activation(a2_sb[:NN, :], ps_u[:NN, :],
                                 mybir.ActivationFunctionType.Relu)
            for s in range(M2):
                ps_t = ps_s.tile([P, NN], f32, tag="ps_small")
                nc.tensor.matmul(ps_t[:], a2_sb[:, bass.ts(s, P)], ident[:, :NN],
                                 start=True, stop=True)
                nc.scalar.copy(a2t_loc[:, s, :], ps_t[:])
                nc.sync.dma_start(cc3_in[s], a2t_loc[:, s, :])
            allgather(cc3_in, cc3_out)
            nc.sync.dma_start(a2T_sb[:], cc3_out[:].rearrange("o p n -> p o n"))

            # ---- L2b (column-parallel) -> reconT slice ----
            for mt in range(M2):
                ps = ps_s.tile([P, NN], f32, tag="ps_small")
                for kt in range(KT):
                    nc.tensor.matmul(ps[:], w2b_sb[:, kt, bass.ts(mt, P)],
                                     a2T_sb[:, kt, :],
                                     start=(kt == 0), stop=(kt == KT - 1))
                nc.scalar.add(recont_sb[:, mt, :], ps[:], b2b_sb[:, mt:mt + 1])
                if recont_cls is not recont_sb:
                    nc.any.tensor_copy(recont_cls[:, mt, :], recont_sb[:, mt, :])
                nc.sync.dma_start(
                    recont_out.ap().rearrange("(o p) n -> o p n", p=P)[mt],
                    recont_sb[:, mt, :])

            # ---- classifier: c1_part = flat_slice @ Wc1_slice  ([1,512] psum) ----
            ps_c1 = ps_c.tile([P, LATENT], f32, tag="ps_c1")
            wc1_view = wc1.ap().rearrange("(n kt p) f -> n p kt f", kt=M2, p=P)
            for n in range(NN):
                chunk = wc1p.tile([P, M2, LATENT], DT_CLS, tag="wc1chunk")
                nc.sync.dma_start(chunk[:], wc1_view[n])
                for kt in range(M2):
                    nc.tensor.matmul(ps_c1[:1, :], recont_cls[:, kt, n:n + 1],
                                     chunk[:, kt, :],
                                     start=(n == 0 and kt == 0),
                                     stop=(n == NN - 1 and kt == M2 - 1))
            # c2_part = c1_part . wc2 + beta
            nc.scalar.copy(c1_sb[:], ps_c1[:1, :])
            nc.vector.tensor_mul(prod_sb[:], c1_sb[:], wc2_sb[:])
            nc.vector.tensor_reduce(c2_sb[:], prod_sb[:],
                                    axis=mybir.AxisListType.X,
                                    op=mybir.AluOpType.add)
            nc.vector.tensor_add(c2_sb[:], c2_sb[:], beta_sb[:])
            nc.sync.dma_start(c2_out.ap(), c2_sb[:])

    nc.compile()
    return nc


_NC_CACHE = {}


def get_nc(reps: int = 1):
    if reps not in _NC_CACHE:
        _NC_CACHE[reps] = build(reps)
    return _NC_CACHE[reps]


def make_in_maps(inputs):
    """Shard the full inputs into 8 per-core input dicts (host side)."""
    z = np.asarray(inputs["latent_z"], np.float32)
    edge = np.asarray(inputs["edge_idx"])
    W1a = np.asarray(inputs["W1a"], np.float32)
    b1a = np.asarray(inputs["b1a"], np.float32)
    W1b = np.asarray(inputs["W1b"], np.float32)
    b1b = np.asarray(inputs["b1b"], np.float32)
    W2a = np.asarray(inputs["W2a"], np.float32)
    b2a = np.asarray(inputs["b2a"], np.float32)
    W2b = np.asarray(inputs["W2b"], np.float32)
    b2b = np.asarray(inputs["b2b"], np.float32)
    Wc1 = np.asarray(inputs["Wc1"], np.float32)
    bc1 = np.asarray(inputs["bc1"], np.float32)
    Wc2 = np.asarray(inputs["Wc2"], np.float32)
    bc2 = np.asarray(inputs["bc2"], np.float32)

    npt = _dt_np(DT_TRUNK)
    npc = _dt_np(DT_CLS)

    # (I + A)^T where A[dst, src] counts edges
    A = np.zeros((NN, NN), np.float64)
    np.add.at(A, (edge[1].astype(np.int64), edge[0].astype(np.int64)), 1.0)
    ahatT = (np.eye(NN) + A).T.astype(np.float32)

    beta = np.array([[(bc1 @ Wc2[:, 0] + bc2[0]) / NCORES]], np.float32)
    wc2_row = Wc2[:, 0][None, :].astype(np.float32)
    Wc1v = Wc1.reshape(NN, NCORES, C2, LATENT)  # [n, core, t', s]

    in_maps = []
    for i in range(NCORES):
        c1s = slice(i * C1, (i + 1) * C1)
        c2s = slice(i * C2, (i + 1) * C2)
        in_maps.append({
            "zin": z.astype(npt),
            "ahatT": ahatT.astype(npt),
            "w1a": np.ascontiguousarray(W1a[:, c1s]).astype(npt),
            "w1b": np.ascontiguousarray(W1b[:, c1s]).astype(npt),
            "w2a": np.ascontiguousarray(W2a[:, c2s]).astype(npt),
            "w2b": np.ascontiguousarray(W2b[:, c2s]).astype(npt),
            "wc1": np.ascontiguousarray(
                Wc1v[:, i].reshape(RCLS, LATENT)).astype(npc),
            "b1a": np.ascontiguousarray(b1a[c1s].reshape(C1 // P, P).T),
            "b1b": np.ascontiguousarray(b1b[c1s].reshape(C1 // P, P).T),
            "b2a": b2a[c2s][None, :].astype(npt),
            "b2b": np.ascontiguousarray(b2b[c2s].reshape(C2 // P, P).T),
            "wc2": wc2_row,
            "beta": beta,
        })
    return in_maps


def run(inputs, reps: int = 1, trace: bool = False):
    nc = get_nc(reps)
    in_maps = make_in_maps(inputs)
    return bass_utils.run_bass_kernel_spmd(
        nc, in_maps, core_ids=list(range(NCORES)), trace=trace)


def kernel(**inputs):
    res = run(inputs, reps=1)
    recont = np.concatenate(
        [res.results[i]["recont_out"] for i in range(NCORES)], axis=0)
    recon = np.ascontiguousarray(recont.T).astype(np.float32)
    logit = sum(float(res.results[i]["c2_out"][0, 0]) for i in range(NCORES))
    dem = np.array([1.0 / (1.0 + np.exp(-logit))], np.float32)
    return (dem, recon)
